# revision 15
# baseline (speedup 1.0000x reference)
"""Trainium2 Bass kernel for nn_AblatedPairEnergies (gnn_message_passing).

Computes, for B=2, L=1024, H=128, O=400, K=30:
  E_idx = knn(CA coords, k=30)                       # [B,L,30] int32
  h_EV  = cat(h_i, h_j, E_nb) @ W_w.T + W_b          # [B,L,30,400] f32
with h_i = V[E_idx[:, :, 0]], h_j = V[E_idx], E_nb = E[b,i,E_idx[b,i,k]].

Strategy (8 NeuronCores, data-parallel over the 2048 (b, i) query rows):
  * Each core owns 256 query rows. Distances are computed on-chip in f32
    (squared distances; sqrt is monotone so the top-k order is identical),
    top-32 per row via 4 rounds of the DVE max8/max_index/match_replace ops.
  * The host stages a fused fp16 table EV[b,i,j] = [E[b,i,j,:] ; V[b,j,:]]
    (512 B rows). One transposed dma_gather per 32-query window fetches
    both the E row and the h_j row feature-major, so they feed the PE
    matmul directly as lhsT with no on-chip transposes. The Q7 descriptor
    generator is the scarce resource (~8-10 ns/idx), so h_i is fetched
    with a tiny 128-idx gather per query block and expanded on the DVE.
  * fp16 x fp16 -> f32 PSUM accumulation keeps rel err ~2.5e-4. The two
    k-halves of an (window, v) group accumulate into one 2-bank PSUM tile
    so a single ACT copy drains both.
"""

import numpy as np

B, L, H, O = 2, 1024, 128, 400
K_NB = 30          # neighbors in the output
K_G = 32           # gathered per query (top-32, padded for alignment)
N_CORES = 8
QPC = (B * L) // N_CORES   # 256 queries per core
NQB = QPC // 128           # query blocks of 128 per core
W_Q = 32                   # queries per EV-gather window (int16 idx range)
EH = H + H                 # fused row width (E + V)

_PROG_CACHE = {}
_PROG_CACHE_LAST = None


def _build_program(has_bias):
    import concourse.bacc as bacc
    import concourse.mybir as mybir
    from concourse import tile
    from concourse.masks import make_identity

    f32 = mybir.dt.float32
    f16 = mybir.dt.float16
    i32 = mybir.dt.int32
    i16 = mybir.dt.int16
    u32 = mybir.dt.uint32
    Square = mybir.ActivationFunctionType.Square

    nc = bacc.Bacc("TRN2", target_bir_lowering=False, debug=False,
                   num_devices=N_CORES)

    EV = nc.dram_tensor("ev", [QPC * L, EH], f16, kind="ExternalInput")
    V_REP = nc.dram_tensor("v_rep", [W_Q * L, H], f16, kind="ExternalInput")
    XQ = nc.dram_tensor("xq", [QPC, 3], f32, kind="ExternalInput")
    XK = nc.dram_tensor("xk", [128, 3 * L], f32, kind="ExternalInput")
    W123 = nc.dram_tensor("w123", [3 * H, O], f16, kind="ExternalInput")
    if has_bias:
        WB = nc.dram_tensor("wb", [128, O], f32, kind="ExternalInput")
    OUT = nc.dram_tensor("h_ev", [QPC, K_G, O], f32, kind="ExternalOutput")
    EIDX = nc.dram_tensor("e_idx", [QPC, K_NB], i32, kind="ExternalOutput")

    NEG_INF = -3.0e38

    with tile.TileContext(nc) as tc:
        with (
            tc.tile_pool(name="const", bufs=1) as const,
            tc.tile_pool(name="work", bufs=2) as work,
            tc.tile_pool(name="gath", bufs=2) as gath,
            tc.tile_pool(name="enb", bufs=4) as enbp,
            tc.tile_pool(name="outp", bufs=4) as outp,
            tc.tile_pool(name="psum_t", bufs=1, space="PSUM") as psum_t,
            tc.tile_pool(name="psum_o", bufs=6, space="PSUM") as psum_o,
        ):
            # ---- constants ----
            ident = const.tile([128, 128], f32)
            make_identity(nc, ident[:])
            xk_t = const.tile([128, 3 * L], f32)
            nc.sync.dma_start(xk_t[:], XK[:])
            w_t = const.tile([128, 3 * O], f16)
            for c in range(3):
                nc.sync.dma_start(w_t[:, c * O:(c + 1) * O],
                                  W123[c * 128:(c + 1) * 128, :])
            # base_f[p] = (p % 32) * 1024  (window-local EV row base)
            base_i = const.tile([128, 1], i32)
            nc.gpsimd.iota(base_i[:], pattern=[[0, 1]], base=0,
                           channel_multiplier=1024)
            sub_i = const.tile([128, 1], i32)
            for g in range(4):
                nc.vector.memset(sub_i[32 * g:32 * (g + 1), :], g * 32768)
            nc.vector.tensor_sub(base_i[:], base_i[:], sub_i[:])
            base_f = const.tile([128, 1], f32)
            nc.vector.tensor_copy(base_f[:], base_i[:])
            if has_bias:
                wb_t = const.tile([128, O], f32)
                nc.sync.dma_start(wb_t[:], WB[:])

            wrs, hixs = [], []
            for qb in range(NQB):
                # ---- squared distances: S[q, j] = |xq_q - xk_j|^2 ----
                xq_t = work.tile([128, 3], f32, tag="xq")
                nc.sync.dma_start(xq_t[:], XQ[qb * 128:(qb + 1) * 128, :])
                xqn = work.tile([128, 3], f32, tag="xqn")
                nc.vector.tensor_scalar_mul(xqn[:], xq_t[:], -1.0)
                sqa = work.tile([128, L], f32, tag="sqa")
                sqb = work.tile([128, L], f32, tag="sqb")
                nc.scalar.activation(sqa[:], xk_t[:, 0:L], Square,
                                     bias=xqn[:, 0:1])
                nc.scalar.activation(sqb[:], xk_t[:, L:2 * L], Square,
                                     bias=xqn[:, 1:2])
                S = work.tile([128, L], f32, tag="S")
                nc.vector.tensor_add(S[:], sqa[:], sqb[:])
                sqc = work.tile([128, L], f32, tag="sqc")
                nc.scalar.activation(sqc[:], xk_t[:, 2 * L:3 * L], Square,
                                     bias=xqn[:, 2:3])
                nc.vector.tensor_add(S[:], S[:], sqc[:])
                negS = work.tile([128, L], f32, tag="negS")
                nc.vector.tensor_scalar_mul(negS[:], S[:], -1.0)

                # ---- top-32 (ascending distance; ties -> lower index) ----
                idxu = work.tile([128, K_G], u32, tag="idxu")
                for r in range(4):
                    vals = work.tile([128, 8], f32, tag="vals")
                    nc.vector.max(out=vals[:], in_=negS[:])
                    nc.vector.max_index(out=idxu[:, 8 * r:8 * (r + 1)],
                                        in_max=vals[:], in_values=negS[:])
                    nc.vector.match_replace(out=negS[:], in_to_replace=vals[:],
                                            in_values=negS[:],
                                            imm_value=NEG_INF)

                eidx = work.tile([128, K_NB], i32, tag="eidx")
                nc.vector.tensor_copy(eidx[:], idxu[:, :K_NB])
                nc.scalar.dma_start(EIDX[qb * 128:(qb + 1) * 128, :], eidx[:])

                # ---- gather-index prep ----
                # vE[q,k] = (q%32)*1024 + idx[q,k]; row 0 doubles as the
                # h_i index stream ((q%32)*1024 + idx[q,0] into V_REP).
                idxf = work.tile([128, K_G], f32, tag="idxf")
                nc.vector.tensor_copy(idxf[:], idxu[:])
                vE = work.tile([128, K_G], f32, tag="vE")
                nc.vector.tensor_scalar_add(vE[:], idxf[:], base_f[:, 0:1])

                tp = psum_t.tile([K_G, 128], f32, tag="tp")
                nc.tensor.transpose(tp[:], vE[:], ident[:])
                ts = work.tile([K_G, 128], i16, tag="ts")
                nc.vector.tensor_copy(ts[:], tp[:])

                # wrapped idx tile: [:, :256] EV windows, [:, 256:264] h_i.
                # position i reads its idx at (partition i%16, slot i//16).
                wr = work.tile([128, 264], i16, tag="wr")
                for w in range(4):
                    for c in range(2):
                        nc.scalar.dma_start(
                            wr[0:16, 64 * w + 32 * c:64 * w + 32 * (c + 1)],
                            ts[16 * c:16 * (c + 1), 32 * w:32 * (w + 1)])
                # h_i stream, plain 2D fold: wrHi[p, s] = ts[0, 8*p + s],
                # so hi-gather position j serves query q = (j%16)*8 + j//16
                # (undone for free in the hiP unpermute below).
                nc.scalar.dma_start(wr[0:16, 256:264], ts[0:1, :])
                # replicate to all 8 gpsimd core groups
                nc.scalar.dma_start(wr[16:32, :], wr[0:16, :])
                nc.scalar.dma_start(wr[32:64, :], wr[0:32, :])
                nc.scalar.dma_start(wr[64:128, :], wr[0:64, :])
                wrs.append(wr)

                # ---- h_i gather + 16x column expansion ----
                hiT = gath.tile([128, 1, 128], f16, tag="hiT")
                nc.gpsimd.dma_gather(
                    out_ap=hiT[:], in_ap=V_REP[:], idxs_ap=wr[:, 256:264],
                    num_idxs=128, num_idxs_reg=128, elem_size=H,
                    transpose=True, single_packet=False)
                hiP = gath.tile([128, 128], f16, tag="hiP")
                nc.vector.tensor_copy(
                    hiP[:].rearrange("f (a b) -> f a b", a=16),
                    hiT[:, 0, :].rearrange("f (b a) -> f a b", a=16))
                hiX = gath.tile([128, 128 * 16], f16, tag="hiX")
                nc.vector.tensor_copy(
                    hiX[:].rearrange("f (q r) -> f q r", r=16),
                    hiP[:].to_broadcast([128, 128, 16]))
                hixs.append(hiX)

            for qb in range(NQB):
                wr, hiX = wrs[qb], hixs[qb]
                for w in range(4):
                    r0 = (qb * 128 + W_Q * w) * L
                    evT = enbp.tile([128, 2, 1024], f16, tag="evT")
                    nc.gpsimd.dma_gather(
                        out_ap=evT[:], in_ap=EV[r0:r0 + W_Q * L, :],
                        idxs_ap=wr[:, 64 * w:64 * (w + 1)],
                        num_idxs=1024, num_idxs_reg=1024, elem_size=EH,
                        transpose=True, single_packet=False)

                    for c in range(2):
                        for v in range(4):
                            col = c * 512 + 128 * v
                            ch = (W_Q * w + 8 * v) * 16
                            ps = psum_o.tile([128, O], f32, tag="ps")
                            nc.tensor.matmul(
                                ps[:], lhsT=hiX[:, ch:ch + 128],
                                rhs=w_t[:, 0:O], start=True, stop=False)
                            nc.tensor.matmul(
                                ps[:], lhsT=evT[:, 1, col:col + 128],
                                rhs=w_t[:, O:2 * O], start=False, stop=False)
                            nc.tensor.matmul(
                                ps[:], lhsT=evT[:, 0, col:col + 128],
                                rhs=w_t[:, 2 * O:3 * O], start=False,
                                stop=True)
                            ob = outp.tile([128, O], f32, tag="ob")
                            if has_bias:
                                nc.vector.tensor_add(ob[:], ps[:], wb_t[:])
                            else:
                                nc.scalar.copy(ob[:], ps[:])
                            q0 = qb * 128 + W_Q * w + 8 * v
                            nc.sync.dma_start(
                                OUT[q0:q0 + 8, 16 * c:16 * (c + 1), :],
                                ob[:])

    nc.compile()
    return nc


def _get_program(has_bias):
    if has_bias not in _PROG_CACHE:
        _PROG_CACHE[has_bias] = _build_program(has_bias)
    return _PROG_CACHE[has_bias]


def _prepare_in_maps(V_embed, E_embed, X, W_w, W_b, has_bias):
    V16 = np.asarray(V_embed, dtype=np.float16)
    E16 = np.asarray(E_embed, dtype=np.float16)
    Xc = np.asarray(X, dtype=np.float32)
    Ww = np.asarray(W_w, dtype=np.float32)
    Wb = np.asarray(W_b, dtype=np.float32)

    Xca = np.ascontiguousarray(Xc[:, :, 1, :])            # [B, L, 3]
    W123 = np.ascontiguousarray(Ww.T.astype(np.float16))  # [384, 400]

    in_maps = []
    for core in range(N_CORES):
        b, qc = divmod(core, N_CORES // B)
        q0 = qc * QPC
        ev = np.empty((QPC, L, EH), np.float16)
        ev[:, :, :H] = E16[b, q0:q0 + QPC]
        ev[:, :, H:] = V16[b][None, :, :]
        v_rep = np.ascontiguousarray(np.tile(V16[b], (W_Q, 1)))
        xq = np.ascontiguousarray(Xca[b, q0:q0 + QPC])
        xk = np.ascontiguousarray(
            np.broadcast_to(Xca[b].T.reshape(1, 3 * L), (128, 3 * L)))
        m = {"ev": ev.reshape(QPC * L, EH), "v_rep": v_rep, "xq": xq,
             "xk": xk, "w123": W123}
        if has_bias:
            m["wb"] = np.ascontiguousarray(np.broadcast_to(Wb, (128, O)))
        in_maps.append(m)
    return in_maps


def kernel(V_embed, E_embed, X, x_mask, chain_idx, W_w, W_b):
    from concourse.bass_utils import run_bass_kernel_spmd

    has_bias = bool(np.any(np.asarray(W_b)))
    nc = _get_program(has_bias)
    in_maps = _prepare_in_maps(V_embed, E_embed, X, W_w, W_b, has_bias)

    global _PROG_CACHE_LAST
    _PROG_CACHE_LAST = (nc, in_maps)
    results = run_bass_kernel_spmd(nc, in_maps, list(range(N_CORES))).results

    h_EV = np.empty((B, L, K_NB, O), np.float32)
    E_idx = np.empty((B, L, K_NB), np.int32)
    for core in range(N_CORES):
        b, qc = divmod(core, N_CORES // B)
        q0 = qc * QPC
        h_EV[b, q0:q0 + QPC] = results[core]["h_ev"][:, :K_NB, :]
        E_idx[b, q0:q0 + QPC] = results[core]["e_idx"]
    return h_EV, E_idx


# revision 20
# speedup vs baseline: 1.1795x; 1.1795x over previous
"""Trainium2 Bass kernel for nn_AblatedPairEnergies (gnn_message_passing).

Computes, for B=2, L=1024, H=128, O=400, K=30:
  E_idx = knn(CA coords, k=30)                       # [B,L,30] int32
  h_EV  = cat(h_i, h_j, E_nb) @ W_w.T + W_b          # [B,L,30,400] f32
with h_i = V[E_idx[:, :, 0]], h_j = V[E_idx], E_nb = E[b,i,E_idx[b,i,k]].

Strategy (8 NeuronCores, data-parallel over the 2048 (b, i) query rows):
  * Each core owns 256 query rows. Distances are computed on-chip in f32
    (squared distances; sqrt is monotone so the top-k order is identical),
    top-32 per row via 4 rounds of the DVE max8/max_index/match_replace ops.
  * The host stages a fused fp16 table EV[b,i,j] = [E[b,i,j,:] ; V[b,j,:]]
    (512 B rows). One transposed dma_gather per 32-query window fetches
    both the E row and the h_j row feature-major, so they feed the PE
    matmul directly as lhsT with no on-chip transposes. The Q7 descriptor
    generator is the scarce resource (~8-10 ns/idx).
  * The gather's int16 index tile (position i%16 -> partition, i//16 ->
    slot, replicated for all 8 Q7 cores) is produced by two tiny K=32
    matmuls against a constant selection matrix - no DMA chains on the
    critical path after top-k.
  * h_i rides for free: the k=0 column of each EV gather's V-half is
    V[E_idx[q,0]]; a small strided-broadcast DVE copy expands it 16x.
  * fp16 x fp16 -> f32 PSUM accumulation keeps rel err ~2.5e-4.
"""

import numpy as np

B, L, H, O = 2, 1024, 128, 400
K_NB = 30          # neighbors in the output
K_G = 32           # gathered per query (top-32, padded for alignment)
N_CORES = 8
QPC = (B * L) // N_CORES   # 256 queries per core
NQB = QPC // 128           # query blocks of 128 per core
W_Q = 32                   # queries per EV-gather window (int16 idx range)
EH = H + H                 # fused row width (E + V)

_PROG_CACHE = {}
_PROG_CACHE_LAST = None


def _build_program(has_bias):
    import concourse.bacc as bacc
    import concourse.mybir as mybir
    from concourse import tile
    from concourse.masks import make_identity

    f32 = mybir.dt.float32
    f16 = mybir.dt.float16
    i32 = mybir.dt.int32
    i16 = mybir.dt.int16
    u32 = mybir.dt.uint32
    Square = mybir.ActivationFunctionType.Square

    nc = bacc.Bacc("TRN2", target_bir_lowering=False, debug=False,
                   num_devices=N_CORES)

    EV = nc.dram_tensor("ev", [QPC * L, EH], f16, kind="ExternalInput")
    XQ = nc.dram_tensor("xq", [QPC, 3], f32, kind="ExternalInput")
    XK = nc.dram_tensor("xk", [128, 3 * L], f32, kind="ExternalInput")
    W123 = nc.dram_tensor("w123", [3 * H, O], f16, kind="ExternalInput")
    SEL = nc.dram_tensor("sel", [32, 256], f32, kind="ExternalInput")
    if has_bias:
        WB = nc.dram_tensor("wb", [128, O], f32, kind="ExternalInput")
    OUT = nc.dram_tensor("h_ev", [QPC, K_G, O], f32, kind="ExternalOutput")
    EIDX = nc.dram_tensor("e_idx", [QPC, K_NB], i32, kind="ExternalOutput")

    NEG_INF = -3.0e38

    with tile.TileContext(nc) as tc:
        with (
            tc.tile_pool(name="const", bufs=1) as const,
            tc.tile_pool(name="work", bufs=2) as work,
            tc.tile_pool(name="gath", bufs=2) as gath,
            tc.tile_pool(name="enb", bufs=4) as enbp,
            tc.tile_pool(name="outp", bufs=4) as outp,
            tc.tile_pool(name="psum_t", bufs=2, space="PSUM") as psum_t,
            tc.tile_pool(name="psum_w", bufs=2, space="PSUM") as psum_w,
            tc.tile_pool(name="psum_o", bufs=4, space="PSUM") as psum_o,
        ):
            # ---- constants ----
            xk_t = const.tile([128, 3 * L], f32)
            nc.sync.dma_start(xk_t[:], XK[:])
            ident = const.tile([128, 128], f32)
            make_identity(nc, ident[:])
            sel_t = const.tile([32, 256], f32)
            nc.sync.dma_start(sel_t[:], SEL[:])
            w_t = const.tile([128, 3 * O], f16)
            for c in range(3):
                nc.sync.dma_start(w_t[:, c * O:(c + 1) * O],
                                  W123[c * 128:(c + 1) * 128, :])
            # base_f[p] = (p % 32) * 1024  (window-local EV row base)
            base_i = const.tile([128, 1], i32)
            nc.gpsimd.iota(base_i[:], pattern=[[0, 1]], base=0,
                           channel_multiplier=1024)
            sub_i = const.tile([128, 1], i32)
            for g in range(4):
                nc.vector.memset(sub_i[32 * g:32 * (g + 1), :], g * 32768)
            nc.vector.tensor_sub(base_i[:], base_i[:], sub_i[:])
            base_f = const.tile([128, 1], f32)
            nc.vector.tensor_copy(base_f[:], base_i[:])
            if has_bias:
                wb_t = const.tile([128, O], f32)
                nc.sync.dma_start(wb_t[:], WB[:])

            for qb in range(NQB):
                # ---- squared distances: S[q, j] = |xq_q - xk_j|^2 ----
                xq_t = work.tile([128, 3], f32, tag="xq")
                nc.sync.dma_start(xq_t[:], XQ[qb * 128:(qb + 1) * 128, :])
                xqn = work.tile([128, 3], f32, tag="xqn")
                nc.vector.tensor_scalar_mul(xqn[:], xq_t[:], -1.0)
                sqa = work.tile([128, L], f32, tag="sqa")
                sqb = work.tile([128, L], f32, tag="sqb")
                nc.scalar.activation(sqa[:], xk_t[:, 0:L], Square,
                                     bias=xqn[:, 0:1])
                nc.scalar.activation(sqb[:], xk_t[:, L:2 * L], Square,
                                     bias=xqn[:, 1:2])
                S = work.tile([128, L], f32, tag="S")
                nc.vector.tensor_add(S[:], sqa[:], sqb[:])
                sqc = work.tile([128, L], f32, tag="sqc")
                nc.scalar.activation(sqc[:], xk_t[:, 2 * L:3 * L], Square,
                                     bias=xqn[:, 2:3])
                nc.vector.tensor_add(S[:], S[:], sqc[:])
                negS = work.tile([128, L], f32, tag="negS")
                nc.vector.tensor_scalar_mul(negS[:], S[:], -1.0)

                # ---- top-32 (ascending distance; ties -> lower index) ----
                idxu = work.tile([128, K_G], u32, tag="idxu")
                for r in range(4):
                    vals = work.tile([128, 8], f32, tag="vals")
                    nc.vector.max(out=vals[:], in_=negS[:])
                    nc.vector.max_index(out=idxu[:, 8 * r:8 * (r + 1)],
                                        in_max=vals[:], in_values=negS[:])
                    nc.vector.match_replace(out=negS[:], in_to_replace=vals[:],
                                            in_values=negS[:],
                                            imm_value=NEG_INF)

                eidx = work.tile([128, K_NB], i32, tag="eidx")
                nc.vector.tensor_copy(eidx[:], idxu[:, :K_NB])
                nc.scalar.dma_start(EIDX[qb * 128:(qb + 1) * 128, :], eidx[:])

                # ---- gather-index prep (no DMAs) ----
                # vE[q,k] = (q%32)*1024 + idx[q,k]; transpose to ts[k, q];
                # wr[p, 64w+32c+q'] = ts[16c + p%16, 32w+q'] via two K=32
                # matmuls with the constant selector (also replicates the
                # wrapped tile across all 8 Q7 core groups).
                idxf = work.tile([128, K_G], f32, tag="idxf")
                nc.vector.tensor_copy(idxf[:], idxu[:])
                vE = work.tile([128, K_G], f32, tag="vE")
                nc.vector.tensor_scalar_add(vE[:], idxf[:], base_f[:, 0:1])

                tp = psum_t.tile([K_G, 128], f32, tag="tp")
                nc.tensor.transpose(tp[:], vE[:], ident[:])
                ts = work.tile([K_G, 128], f32, tag="ts")
                nc.vector.tensor_copy(ts[:], tp[:])

                wr = work.tile([128, 256], i16, tag="wr")
                for c in range(2):
                    wrp = psum_w.tile([128, 128], f32, tag="wrp")
                    nc.tensor.matmul(wrp[:],
                                     lhsT=sel_t[:, 128 * c:128 * (c + 1)],
                                     rhs=ts[:], start=True, stop=True)
                    nc.vector.tensor_copy(
                        wr[:].rearrange("p (w x) -> p w x",
                                        w=4)[:, :, 32 * c:32 * (c + 1)],
                        wrp[:].rearrange("p (w q) -> p w q", w=4))

                # ---- gathers + matmuls ----
                for w in range(4):
                    r0 = (qb * 128 + W_Q * w) * L
                    evT = enbp.tile([128, 2, 1024], f16, tag="evT")
                    nc.gpsimd.dma_gather(
                        out_ap=evT[:], in_ap=EV[r0:r0 + W_Q * L, :],
                        idxs_ap=wr[:, 64 * w:64 * (w + 1)],
                        num_idxs=1024, num_idxs_reg=1024, elem_size=EH,
                        transpose=True, single_packet=False)

                    # h_i for this window: V-half of the k=0 columns,
                    # expanded 16x along the pair axis.
                    hiX = gath.tile([128, 512], f16, tag="hiX")
                    nc.vector.tensor_copy(
                        hiX[:].rearrange("f (q r) -> f q r", r=16),
                        evT[:, 1, 0:512].rearrange("f (q s) -> f q s",
                                                   s=16)[:, :, 0:1]
                        .to_broadcast([128, 32, 16]))

                    for c in range(2):
                        for v in range(4):
                            col = c * 512 + 128 * v
                            ps = psum_o.tile([128, O], f32, tag="ps")
                            nc.tensor.matmul(
                                ps[:], lhsT=evT[:, 0, col:col + 128],
                                rhs=w_t[:, 2 * O:3 * O], start=True,
                                stop=False)
                            nc.tensor.matmul(
                                ps[:], lhsT=evT[:, 1, col:col + 128],
                                rhs=w_t[:, O:2 * O], start=False, stop=False)
                            nc.tensor.matmul(
                                ps[:], lhsT=hiX[:, 128 * v:128 * (v + 1)],
                                rhs=w_t[:, 0:O], start=False, stop=True)
                            ob = outp.tile([128, O], f32, tag="ob")
                            if has_bias:
                                nc.vector.tensor_add(ob[:], ps[:], wb_t[:])
                            else:
                                nc.scalar.copy(ob[:], ps[:])
                            q0 = qb * 128 + W_Q * w + 8 * v
                            nc.sync.dma_start(
                                OUT[q0:q0 + 8, 16 * c:16 * (c + 1), :],
                                ob[:])

    nc.compile()
    return nc


def _get_program(has_bias):
    if has_bias not in _PROG_CACHE:
        _PROG_CACHE[has_bias] = _build_program(has_bias)
    return _PROG_CACHE[has_bias]


def _make_sel():
    sel = np.zeros((32, 256), np.float32)
    for c in range(2):
        for k in range(32):
            if k // 16 == c:
                sel[k, 128 * c + (k % 16)::16][:8] = 1.0
    return sel


def _prepare_in_maps(V_embed, E_embed, X, W_w, W_b, has_bias):
    V16 = np.asarray(V_embed, dtype=np.float16)
    E16 = np.asarray(E_embed, dtype=np.float16)
    Xc = np.asarray(X, dtype=np.float32)
    Ww = np.asarray(W_w, dtype=np.float32)
    Wb = np.asarray(W_b, dtype=np.float32)

    Xca = np.ascontiguousarray(Xc[:, :, 1, :])            # [B, L, 3]
    W123 = np.ascontiguousarray(Ww.T.astype(np.float16))  # [384, 400]
    sel = _make_sel()

    in_maps = []
    for core in range(N_CORES):
        b, qc = divmod(core, N_CORES // B)
        q0 = qc * QPC
        ev = np.empty((QPC, L, EH), np.float16)
        ev[:, :, :H] = E16[b, q0:q0 + QPC]
        ev[:, :, H:] = V16[b][None, :, :]
        xq = np.ascontiguousarray(Xca[b, q0:q0 + QPC])
        xk = np.ascontiguousarray(
            np.broadcast_to(Xca[b].T.reshape(1, 3 * L), (128, 3 * L)))
        m = {"ev": ev.reshape(QPC * L, EH), "xq": xq, "xk": xk,
             "w123": W123, "sel": sel}
        if has_bias:
            m["wb"] = np.ascontiguousarray(np.broadcast_to(Wb, (128, O)))
        in_maps.append(m)
    return in_maps


def kernel(V_embed, E_embed, X, x_mask, chain_idx, W_w, W_b):
    from concourse.bass_utils import run_bass_kernel_spmd

    has_bias = bool(np.any(np.asarray(W_b)))
    nc = _get_program(has_bias)
    in_maps = _prepare_in_maps(V_embed, E_embed, X, W_w, W_b, has_bias)

    global _PROG_CACHE_LAST
    _PROG_CACHE_LAST = (nc, in_maps)
    results = run_bass_kernel_spmd(nc, in_maps, list(range(N_CORES))).results

    h_EV = np.empty((B, L, K_NB, O), np.float32)
    E_idx = np.empty((B, L, K_NB), np.int32)
    for core in range(N_CORES):
        b, qc = divmod(core, N_CORES // B)
        q0 = qc * QPC
        h_EV[b, q0:q0 + QPC] = results[core]["h_ev"][:, :K_NB, :]
        E_idx[b, q0:q0 + QPC] = results[core]["e_idx"]
    return h_EV, E_idx
